# revision 21
# baseline (speedup 1.0000x reference)
"""Trainium2 Bass kernel for nn_Comb5 (gnn_message_passing) — v2.

Data-parallel over batch: 32 batches -> 8 cores x 4 batches.
Key structure vs v1:
 - conv tap tiles (ED2) built by one DVE multiply against host-precomputed
   shifted-weight tiles (Wshift) instead of 56 PE matmuls + 50 narrow copies.
 - A_raw computed block-diagonally (per-t 32x49 matmuls, col-group packed).
 - feat1^T computed directly on PE from a compact block-diag Asm^T (ATP),
   no gpsimd DMAs / padded AsmPT.
 - branch1 LN row-sums fused into the PSUM->SBUF drain on ScalarE
   (activation Identity + accum_out); sumsq via tensor_tensor_reduce bf16.
 - rownorms via bn_stats + small wide combines; one Sqrt site per use.
 - single Sigmoid at end of program; weights pre-transposed on host.
"""

import sys

sys.path.insert(0, "/opt/trn_rl_repo")

import numpy as np
import ml_dtypes

from concourse import bass, bacc, tile, mybir

f32 = mybir.dt.float32
f32r = mybir.dt.float32r
bf16 = mybir.dt.bfloat16
AX = mybir.AxisListType
OP = mybir.AluOpType
AF = mybir.ActivationFunctionType

B, T, N, C, BG, POSD, KK = 32, 16, 32, 256, 49, 9, 7
TN = T * N          # 512
OT = T - KK + 1     # 10
NCORES = 8


def jcs_of(ot):
    return list(range(ot // 4, min(3, (ot + 6) // 4) + 1))


# (jc, p) pairs for the pair-packed conv tap tiles
PAIRS = []
for p in range(OT // 2):
    for jc in sorted(set(jcs_of(2 * p)) | set(jcs_of(2 * p + 1))):
        PAIRS.append((jc, p))
PAIR_IDX = {k: i for i, k in enumerate(PAIRS)}
NPAIR = len(PAIRS)  # 12

# conv matmul groups: per och chunk, list of (p, [jc...])
OCH_GROUPS = []
for och in range(3):
    prs = list(range(och * 2, min(OT // 2, och * 2 + 2)))
    OCH_GROUPS.append([(p, sorted(set(jcs_of(2 * p)) | set(jcs_of(2 * p + 1))))
                       for p in prs])


def _r(ap):
    return ap.bitcast(f32r)


def build_nc(bpc, trivial_gb1, trivial_gb2, use_lrelu_act=True, stage=9):
    nc = bacc.Bacc(target_bir_lowering=False, debug=False)

    lf_d = nc.declare_dram_parameter("local_feat", [bpc, T, N, C], f32, isOutput=False)
    gf_d = nc.declare_dram_parameter("global_feat", [bpc, T, BG, C], f32, isOutput=False)
    pos_d = nc.declare_dram_parameter("pos", [bpc, T, N, POSD], f32, isOutput=False)
    w1_d = nc.declare_dram_parameter("tc_adj_w", [C, C], f32, isOutput=False)
    w2_d = nc.declare_dram_parameter("bi_adj_w", [C, C], f32, isOutput=False)
    wsh_d = nc.declare_dram_parameter("wshift", [NPAIR, 128, 2 * C], f32, isOutput=False)
    wafft_d = nc.declare_dram_parameter("wafft", [3, 128, C], f32, isOutput=False)
    baff_d = nc.declare_dram_parameter("bi_aff_b", [1, C], f32, isOutput=False)
    wredt_d = nc.declare_dram_parameter("wredt", [4, 128, C], f32, isOutput=False)
    bred_d = nc.declare_dram_parameter("red_b", [1, C], f32, isOutput=False)
    ident_d = nc.declare_dram_parameter("ident", [128, 128], f32, isOutput=False)
    identb_d = nc.declare_dram_parameter("identb", [128, 128], f32, isOutput=False)
    wacol_d = nc.declare_dram_parameter("wa_col", [128, 4], f32, isOutput=False)
    wprep_d = nc.declare_dram_parameter("wp_rep", [128, POSD], f32, isOutput=False)
    batt_d = nc.declare_dram_parameter("batt_rep", [128, 1], f32, isOutput=False)
    g1_d = nc.declare_dram_parameter("g1r", [128, C], f32, isOutput=False)
    b1_d = nc.declare_dram_parameter("b1r", [128, C], f32, isOutput=False)
    g2_d = nc.declare_dram_parameter("g2r", [128, C], f32, isOutput=False)
    b2_d = nc.declare_dram_parameter("b2r", [128, C], f32, isOutput=False)
    out_d = nc.declare_dram_parameter("out", [bpc, T, N, C], f32, isOutput=True)

    dma = nc.sync.dma_start

    with tile.TileContext(nc) as tc:
        with (
            tc.tile_pool(name="const", bufs=1) as cpool,
            tc.tile_pool(name="work", bufs=1) as wpool,
            tc.tile_pool(name="scr", bufs=2) as spool,
            tc.tile_pool(name="ps", bufs=1, space="PSUM") as psp,
        ):
            def psA(dt=f32):
                return psp.tile([128, 512], dt, tag="pA", name="pA", bufs=2)

            def psB(dt=f32):
                return psp.tile([128, 1024], dt, tag="pB", name="pB", bufs=2)

            def psC(dt=f32):
                return psp.tile([128, 512], dt, tag="pC", name="pC", bufs=2)

            # ---------------- constants (pure DMA) ----------------
            ident = cpool.tile([128, 128], f32)
            dma(ident[:], ident_d[:])
            identb = cpool.tile([128, 128], bf16)
            idld = spool.tile([128, 128], f32, tag="idld", name="idld")
            dma(idld[:], identb_d[:])
            nc.scalar.copy(identb[:], idld[:])
            wsh = [cpool.tile([128, 2 * C], bf16, tag=f"wsh{i}", name=f"wsh{i}")
                   for i in range(NPAIR)]
            for i in range(NPAIR):
                wshld = spool.tile([128, 2 * C], f32, tag="wshld", name="wshld")
                dma(wshld[:], wsh_d[i])
                (nc.scalar.copy if i % 2 else nc.vector.tensor_copy)(wsh[i][:], wshld[:])
            w1 = [cpool.tile([128, C], f32, tag=f"w1_{kc}", name=f"w1_{kc}") for kc in range(2)]
            w2 = [cpool.tile([128, C], f32, tag=f"w2_{kc}", name=f"w2_{kc}") for kc in range(2)]
            for kc in range(2):
                wld = spool.tile([128, C], f32, tag="wld", name="wld")
                dma(wld[:], w1_d[kc * 128 : kc * 128 + 128, :])
                nc.scalar.copy(w1[kc][:].bitcast(f32r), wld[:])
                wld2 = spool.tile([128, C], f32, tag="wld2", name="wld2")
                dma(wld2[:], w2_d[kc * 128 : kc * 128 + 128, :])
                nc.scalar.copy(w2[kc][:].bitcast(f32r), wld2[:])
            wafft = [cpool.tile([128, C], bf16, tag=f"wafft{j}", name=f"wafft{j}") for j in range(3)]
            for j in range(3):
                wald = spool.tile([128, C], f32, tag="wald", name="wald")
                dma(wald[:], wafft_d[j])
                nc.vector.tensor_copy(wafft[j][:], wald[:])
            wredt = [cpool.tile([128, C], bf16, tag=f"wredt{j}", name=f"wredt{j}") for j in range(4)]
            for j in range(4):
                wrld = spool.tile([128, C], f32, tag="wrld", name="wrld")
                dma(wrld[:], wredt_d[j])
                nc.scalar.copy(wredt[j][:], wrld[:])
            wa_col = cpool.tile([128, 4], f32)
            dma(wa_col[:], wacol_d[:])
            wp_rep = cpool.tile([128, POSD], f32)
            dma(wp_rep[:], wprep_d[:])
            batt_rep = cpool.tile([128, 1], f32)
            dma(batt_rep[:], batt_d[:])
            baff_row = cpool.tile([1, C], f32)
            dma(baff_row[:], baff_d[:])
            bred_row = cpool.tile([1, C], f32)
            dma(bred_row[:], bred_d[:])
            onesb = cpool.tile([1, 128], bf16)
            nc.vector.memset(onesb[:], 1.0)
            g1r = b1r = g2r = b2r = None
            if not trivial_gb1:
                g1r = cpool.tile([128, C], f32, tag="g1r", name="g1r")
                b1r = cpool.tile([128, C], f32, tag="b1r", name="b1r")
                dma(g1r[:], g1_d[:])
                dma(b1r[:], b1_d[:])
            if not trivial_gb2:
                g2r = cpool.tile([128, C], f32, tag="g2r", name="g2r")
                b2r = cpool.tile([128, C], f32, tag="b2r", name="b2r")
                dma(g2r[:], g2_d[:])
                dma(b2r[:], b2_d[:])
            # aff / red bias rows in bf16 (added via K=1 matmul into accum groups)
            baff_b = cpool.tile([1, C], bf16)
            bred_b = cpool.tile([1, C], bf16)
            nc.vector.tensor_copy(baff_b[:], baff_row[:])
            nc.vector.tensor_copy(bred_b[:], bred_row[:])

            # persistent
            pos_w = [wpool.tile([128, bpc * POSD], f32, tag=f"pos{i}", name=f"pos{i}")
                     for i in range(4)]
            red_sb = [wpool.tile([128, bpc * C], bf16, tag=f"red{i}", name=f"red{i}")
                      for i in range(4)]
            garg_w = wpool.tile([128, 4 * bpc], f32)
            rinv_w = wpool.tile([128, 4 * bpc], f32)

            # ATP: persistent, zero pads written once
            atp = wpool.tile([128, 512], bf16, tag="ATP", name="ATP")
            nc.vector.memset(atp[:], 0.0)
            # gf tiles persistent: pad rows (49:64, 113:128) must stay finite
            gf = [wpool.tile([128, C], f32, tag=f"gf{h}", name=f"gf{h}") for h in range(8)]
            for h in range(8):
                nc.vector.memset(gf[h][:], 1.0)

            # ---------------- per batch ----------------
            for b in range(bpc):
                lfb_dram = lf_d[b].flatten_outer_dims()
                gfb_dram = gf_d[b].flatten_outer_dims()
                posb_dram = pos_d[b].flatten_outer_dims()
                outb = out_d[b].flatten_outer_dims()

                # S1: loads
                lf = [spool.tile([128, C], f32, tag=f"lf{i}", name=f"lf{i}") for i in range(4)]
                for ic in range(4):
                    dma(lf[ic][:], lfb_dram[ic * 128 : ic * 128 + 128, :])
                for t in range(T):
                    dma(gf[t // 2][64 * (t % 2) : 64 * (t % 2) + BG, :],
                        gfb_dram[t * BG : t * BG + BG, :])
                for ic in range(4):
                    dma(pos_w[ic][:, b * POSD : (b + 1) * POSD],
                        posb_dram[ic * 128 : ic * 128 + 128, :])

                # S2: lf rownorm -> nf, lfb
                # bn_stats layout per op: [cnt1, m1, M2_1, cnt2, m2, M2_2]
                # ssq = M2a+M2b + 128*(ma^2+mb^2)
                stl = spool.tile([128, 24], f32, tag="stl", name="stl")
                for ic in range(4):
                    nc.vector.bn_stats(stl[:, ic * 6 : ic * 6 + 6], lf[ic][:])
                stl4 = stl[:].rearrange("p (i g v) -> p i g v", i=4, g=2)  # v=3
                msq = spool.tile([128, 8], f32, tag="msq", name="msq")
                msq3 = msq[:].rearrange("p (i g) -> p i g", i=4)
                nc.vector.tensor_tensor(
                    msq3, stl4[:, :, :, 1], stl4[:, :, :, 1], op=OP.mult)
                m2s = spool.tile([128, 8], f32, tag="m2s", name="m2s")
                nc.vector.scalar_tensor_tensor(
                    m2s[:].rearrange("p (i g) -> p i g", i=4),
                    msq3, 128.0, stl4[:, :, :, 2], op0=OP.mult, op1=OP.add)
                ssqA = spool.tile([128, 12], f32, tag="ssqA", name="ssqA")
                nc.vector.tensor_reduce(
                    ssqA[:, 0:4], m2s[:].rearrange("p (i g) -> p i g", i=4),
                    axis=AX.X, op=OP.add)
                # S3: gf rownorm -> nfg (pads garbage, never read)
                stg = spool.tile([128, 48], f32, tag="stg", name="stg")
                for h in range(8):
                    nc.vector.bn_stats(stg[:, h * 6 : h * 6 + 6], gf[h][:])
                stg4 = stg[:].rearrange("p (i g v) -> p i g v", i=8, g=2)
                msqg = spool.tile([128, 16], f32, tag="msqg", name="msqg")
                msqg3 = msqg[:].rearrange("p (i g) -> p i g", i=8)
                nc.vector.tensor_tensor(
                    msqg3, stg4[:, :, :, 1], stg4[:, :, :, 1], op=OP.mult)
                m2sg = spool.tile([128, 16], f32, tag="m2sg", name="m2sg")
                nc.vector.scalar_tensor_tensor(
                    m2sg[:].rearrange("p (i g) -> p i g", i=8),
                    msqg3, 128.0, stg4[:, :, :, 2], op0=OP.mult, op1=OP.add)
                nc.vector.tensor_reduce(
                    ssqA[:, 4:12], m2sg[:].rearrange("p (i g) -> p i g", i=8),
                    axis=AX.X, op=OP.add)
                nrmA = spool.tile([128, 12], f32, tag="nrmA", name="nrmA")
                nc.scalar.sqrt(nrmA[:], ssqA[:])
                invA = spool.tile([128, 12], f32, tag="invA", name="invA")
                nc.vector.reciprocal(invA[:], nrmA[:])
                inv4 = invA[:, 0:4]
                inv8 = invA[:, 4:12]
                nf = [spool.tile([128, C], f32, tag=f"nf{i}", name=f"nf{i}", bufs=1) for i in range(4)]
                lfb = [spool.tile([128, C], bf16, tag=f"lfb{i}", name=f"lfb{i}") for i in range(4)]
                for ic in range(4):
                    nc.vector.tensor_scalar_mul(nf[ic][:], lf[ic][:], inv4[:, ic : ic + 1])
                    nc.scalar.copy(lfb[ic][:], lf[ic][:])
                nfg = [spool.tile([128, C], bf16, tag=f"nfg{h}", name=f"nfg{h}", bufs=1) for h in range(8)]
                gfr = [spool.tile([128, C], bf16, tag=f"gfr{h}", name=f"gfr{h}") for h in range(8)]
                for h in range(8):
                    nc.vector.tensor_scalar_mul(nfg[h][:], gf[h][:], inv8[:, h : h + 1])
                    nc.scalar.copy(gfr[h][:], gf[h][:])

                # S4: transposes
                nfT = [spool.tile([128, TN], f32, tag=f"nfT{cc}", name=f"nfT{cc}") for cc in range(2)]
                for cc in range(2):
                    pt = psC()
                    for jc in range(4):
                        nc.tensor.transpose(
                            pt[:, jc * 128 : jc * 128 + 128],
                            nf[jc][:, cc * 128 : cc * 128 + 128], ident[:])
                    (nc.scalar.copy if cc == 0 else nc.vector.tensor_copy)(
                        nfT[cc][:].bitcast(f32r), pt[:])
                nfgT = [spool.tile([128, 1024], bf16, tag=f"nfgT{cc}", name=f"nfgT{cc}", bufs=1)
                        for cc in range(2)]
                for cc in range(2):
                    pt = psB(bf16)
                    for h in range(8):
                        nc.tensor.transpose(
                            pt[:, h * 128 : h * 128 + 128],
                            nfg[h][:, cc * 128 : cc * 128 + 128], identb[:])
                    (nc.scalar.copy if cc == 0 else nc.vector.tensor_copy)(
                        nfgT[cc][:], pt[:])

                if stage <= 2:
                    for ic in range(4):
                        dma(outb[ic * 128 : ic * 128 + 128, :], nf[ic][:])
                    continue

                # S5: branch1 A chain -> Ind -> IndT
                ut = [spool.tile([128, TN], f32, tag=f"ut{cc}", name=f"ut{cc}") for cc in range(2)]
                for cc in range(2):
                    pu = psA()
                    for kc in range(2):
                        nc.tensor.matmul(
                            pu[:], _r(w1[kc][:, cc * 128 : cc * 128 + 128]), _r(nfT[kc][:]),
                            start=(kc == 0), stop=(kc == 1))
                    (nc.scalar.copy if cc == 0 else nc.vector.tensor_copy)(
                        ut[cc][:].bitcast(f32r), pu[:])
                Ind = [spool.tile([128, TN], bf16, tag=f"ind{ic}", name=f"ind{ic}") for ic in range(4)]
                for ic in range(4):
                    pa = psA()
                    for kc in range(2):
                        nc.tensor.matmul(
                            pa[:], _r(ut[kc][:, ic * 128 : ic * 128 + 128]), _r(nfT[kc][:]),
                            start=(kc == 0), stop=(kc == 1))
                    bmax = spool.tile([128, T], f32, tag="bmax", name="bmax")
                    nc.vector.tensor_reduce(
                        bmax[:], pa[:].rearrange("p (t n) -> p t n", t=T),
                        axis=AX.X, op=OP.max)
                    nc.vector.tensor_tensor(
                        Ind[ic][:].rearrange("p (t n) -> p t n", t=T),
                        pa[:].rearrange("p (t n) -> p t n", t=T),
                        bmax[:].unsqueeze(2).broadcast_to([128, T, N]),
                        op=OP.is_equal)

                if stage <= 3:
                    for ic in range(4):
                        scc = spool.tile([128, C], f32, tag="stgc", name="stgc")
                        nc.scalar.copy(scc[:], Ind[ic][:, 0:C])
                        dma(outb[ic * 128 : ic * 128 + 128, :], scc[:])
                    continue

                IndT = [spool.tile([128, TN], bf16, tag=f"indT{jc}", name=f"indT{jc}")
                        for jc in range(4)]
                for jc in range(4):
                    pt = psC(bf16)
                    for ic in range(4):
                        nc.tensor.transpose(
                            pt[:, ic * 128 : ic * 128 + 128],
                            Ind[ic][:, jc * 128 : jc * 128 + 128], identb[:])
                    (nc.vector.tensor_copy if jc % 2 else nc.scalar.copy)(IndT[jc][:], pt[:])

                # S5.5: ED2 tiles via one DVE mult each
                ED2 = {}
                for (jc, p) in PAIRS:
                    i = PAIR_IDX[(jc, p)]
                    tl_ = spool.tile([128, 2 * C], bf16, tag=f"ed2_{i}", name=f"ed2_{i}", bufs=1)
                    nc.vector.tensor_tensor(
                        tl_[:].rearrange("p (h c) -> p h c", h=2),
                        lfb[jc][:].unsqueeze(1).broadcast_to([128, 2, C]),
                        wsh[i][:].rearrange("p (h c) -> p h c", h=2),
                        op=OP.mult)
                    ED2[(jc, p)] = tl_

                # S6: conv -> y (+s1 on drain), s2, LN combine, z (lrelu), lf1
                s1w = spool.tile([128, 40], f32, tag="s1w", name="s1w")
                s2w = spool.tile([128, 40], f32, tag="s2w", name="s2w")
                y_sb = [spool.tile([128, OT * C], bf16, tag=f"y{ic}", name=f"y{ic}", bufs=1)
                        for ic in range(4)]
                for ic in range(4):
                    for och in range(3):
                        # jc-outer so the IndT stationary is reused across p's
                        groups = OCH_GROUPS[och]
                        seq = []
                        for pi, (p, pjcs) in enumerate(groups):
                            for ji, jc in enumerate(pjcs):
                                seq.append((jc, pi, p, ji == 0, ji == len(pjcs) - 1))
                        seq.sort(key=lambda x: x[0])
                        py = psB()
                        for jc, pi, p, st_, sp_ in seq:
                            nc.tensor.matmul(
                                py[:, pi * 2 * C : pi * 2 * C + 2 * C],
                                IndT[jc][:, ic * 128 : ic * 128 + 128],
                                ED2[(jc, p)][:],
                                start=st_, stop=sp_)
                        n_el = 2 * len(groups) * C
                        nc.scalar.copy(
                            y_sb[ic][:, och * 4 * C : och * 4 * C + n_el],
                            py[:, 0:n_el])
                    nc.vector.tensor_reduce(
                        s1w[:, ic * 10 : ic * 10 + 10],
                        y_sb[ic][:].rearrange("p (o c) -> p o c", o=OT),
                        axis=AX.X, op=OP.add)
                    for ot in range(OT):
                        sl = slice(ot * C, (ot + 1) * C)
                        scr2 = spool.tile([128, C], bf16, tag="sqscr", name="sqscr")
                        nc.vector.scalar_tensor_tensor(
                            scr2[:], y_sb[ic][:, sl], 1.0, y_sb[ic][:, sl],
                            op0=OP.mult, op1=OP.mult,
                            accum_out=s2w[:, ic * 10 + ot : ic * 10 + ot + 1])
                if stage == 4:
                    for ic in range(4):
                        scc4 = spool.tile([128, C], f32, tag="stgc", name="stgc")
                        nc.scalar.copy(scc4[:], y_sb[ic][:, 0:C])
                        dma(outb[ic * 128 : ic * 128 + 128, :], scc4[:])
                    continue
                muw = spool.tile([128, 40], f32, tag="muw", name="muw")
                nc.vector.tensor_scalar_mul(muw[:], s1w[:], 1.0 / 256.0)
                exw = spool.tile([128, 40], f32, tag="exw", name="exw")
                nc.vector.tensor_scalar_mul(exw[:], s2w[:], 1.0 / 256.0)
                msqw = spool.tile([128, 40], f32, tag="msqw", name="msqw")
                nc.vector.tensor_tensor(msqw[:], muw[:], muw[:], op=OP.mult)
                nc.vector.tensor_tensor(exw[:], exw[:], msqw[:], op=OP.subtract)
                nc.vector.tensor_scalar_add(exw[:], exw[:], 1e-5)
                sdw = spool.tile([128, 40], f32, tag="sdw", name="sdw")
                nc.scalar.sqrt(sdw[:], exw[:])
                alw = spool.tile([128, 40], f32, tag="alw", name="alw")
                nc.vector.reciprocal(alw[:], sdw[:])
                bew = spool.tile([128, 40], f32, tag="bew", name="bew")
                nc.vector.scalar_tensor_tensor(
                    bew[:], muw[:], -1.0, alw[:], op0=OP.mult, op1=OP.mult)
                lf1b = [spool.tile([128, C], bf16, tag=f"lf1b{ic}", name=f"lf1b{ic}")
                        for ic in range(4)]
                for ic in range(4):
                    for ot in range(OT):
                        sl = slice(ot * C, (ot + 1) * C)
                        col = slice(ic * 10 + ot, ic * 10 + ot + 1)
                        if trivial_gb1 and use_lrelu_act:
                            nc.scalar.activation(
                                y_sb[ic][:, sl], y_sb[ic][:, sl], AF.Lrelu,
                                bias=bew[:, col], scale=alw[:, col], alpha=0.01)
                        else:
                            nc.scalar.activation(
                                y_sb[ic][:, sl], y_sb[ic][:, sl], AF.Identity,
                                bias=bew[:, col], scale=alw[:, col])
                            if not trivial_gb1:
                                nc.vector.tensor_tensor(
                                    y_sb[ic][:, sl], y_sb[ic][:, sl], g1r[:], op=OP.mult)
                                nc.vector.tensor_tensor(
                                    y_sb[ic][:, sl], y_sb[ic][:, sl], b1r[:], op=OP.add)
                            nc.vector.scalar_tensor_tensor(
                                y_sb[ic][:, sl], y_sb[ic][:, sl], 0.01,
                                y_sb[ic][:, sl], op0=OP.mult, op1=OP.max)
                    # mean over ot: pairwise tree (z viewed (p, 5, 2, 256))
                    zv = y_sb[ic][:].rearrange("p (u e c) -> p u e c", u=5, e=2)
                    mtr = spool.tile([128, 5 * C], bf16, tag="mtr", name="mtr")
                    nc.vector.tensor_tensor(
                        mtr[:].rearrange("p (u c) -> p u c", u=5),
                        zv[:, :, 0], zv[:, :, 1], op=OP.add)
                    mv = mtr[:].rearrange("p (u c) -> p u c", u=5)
                    m2 = spool.tile([128, 2 * C], bf16, tag="mt2", name="mt2")
                    nc.vector.tensor_tensor(
                        m2[:].rearrange("p (u c) -> p u c", u=2),
                        mv[:, 0:2], mv[:, 2:4], op=OP.add)
                    m3 = spool.tile([128, C], bf16, tag="mt3", name="mt3")
                    nc.vector.tensor_tensor(
                        m3[:], m2[:, 0:C], m2[:, C : 2 * C], op=OP.add)
                    nc.vector.tensor_tensor(m3[:], m3[:], mv[:, 4], op=OP.add)
                    nc.vector.tensor_scalar_mul(lf1b[ic][:], m3[:], 1.0 / OT)

                if stage <= 5:
                    for ic in range(4):
                        scc = spool.tile([128, C], f32, tag="stgc", name="stgc")
                        nc.scalar.copy(scc[:], lf1b[ic][:])
                        dma(outb[ic * 128 : ic * 128 + 128, :], scc[:])
                    continue

                # S7: branch2 A chain (block-diag) -> softmax
                ut2 = [spool.tile([128, TN], bf16, tag=f"ut2{cc}", name=f"ut2{cc}") for cc in range(2)]
                for cc in range(2):
                    pu = psA()
                    for kc in range(2):
                        nc.tensor.matmul(
                            pu[:], _r(w2[kc][:, cc * 128 : cc * 128 + 128]), _r(nfT[kc][:]),
                            start=(kc == 0), stop=(kc == 1))
                    (nc.scalar.copy if cc == 0 else nc.vector.tensor_copy)(
                        ut2[cc][:], pu[:])
                Araw = [spool.tile([128, BG], f32, tag=f"araw{ic}", name=f"araw{ic}")
                        for ic in range(4)]
                Asm = [spool.tile([128, BG], bf16, tag=f"asm{ic}", name=f"asm{ic}")
                       for ic in range(4)]
                den_scr = spool.tile([128, BG], f32, tag="denscr", name="denscr")
                for ic in range(4):
                    pa = psA()
                    for tl in range(4):
                        t_g = ic * 4 + tl
                        for kc in range(2):
                            nc.tensor.matmul(
                                pa[32 * tl : 32 * tl + 32, 0:BG],
                                ut2[kc][:, t_g * 32 : t_g * 32 + 32],
                                nfgT[kc][:, t_g * 64 : t_g * 64 + BG],
                                start=(kc == 0), stop=(kc == 1),
                                tile_position=(0, 32 * tl))
                    nc.vector.tensor_copy(Araw[ic][:], pa[:, 0:BG])
                    rmax = spool.tile([128, 1], f32, tag="rmax", name="rmax")
                    nc.vector.tensor_reduce(rmax[:], Araw[ic][:], axis=AX.X, op=OP.max)
                    nbias = spool.tile([128, 1], f32, tag="nbias", name="nbias")
                    nc.vector.tensor_scalar_mul(nbias[:], rmax[:], -5.0)
                    den = spool.tile([128, 1], f32, tag="smden", name="smden")
                    nc.scalar.activation(
                        den_scr[:], Araw[ic][:], AF.Exp, bias=nbias[:], scale=5.0,
                        accum_out=den[:])
                    rden = spool.tile([128, 1], f32, tag="smrden", name="smrden")
                    nc.vector.reciprocal(rden[:], den[:])
                    nc.vector.tensor_scalar_mul(Asm[ic][:], den_scr[:], rden[:])

                # S8: ATP (block-diag Asm^T) via sub-transposes
                for ic in range(4):
                    ptb = psC(bf16)
                    for tl in range(4):
                        po = 64 * (tl % 2)
                        nc.tensor.transpose(
                            ptb[po : po + BG, 32 * tl : 32 * tl + 32],
                            Asm[ic][32 * tl : 32 * tl + 32, :],
                            identb[32 * tl : 32 * tl + 32, 32 * tl : 32 * tl + 32],
                            tile_position=(32 * tl, po))
                    # merged copies: even tl -> rows 0:49 (i-even slot), odd -> rows 64:113
                    # psum cols 32*tl -> (pair a, parity e, 32); atp cols h*64+32*par
                    pv4 = ptb[0:BG, 0:128].rearrange("p (a e c) -> p a e c", a=2, e=2)
                    av4 = atp[0:BG, 128 * ic : 128 * ic + 128].rearrange(
                        "p (h e c) -> p h e c", h=2, e=2)
                    nc.vector.tensor_copy(av4[:, :, 0], pv4[:, :, 0])
                    pv4o = ptb[64 : 64 + BG, 0:128].rearrange("p (a e c) -> p a e c", a=2, e=2)
                    av4o = atp[64 : 64 + BG, 128 * ic : 128 * ic + 128].rearrange(
                        "p (h e c) -> p h e c", h=2, e=2)
                    nc.vector.tensor_copy(av4o[:, :, 1], pv4o[:, :, 1])

                # S9: feat1^T, ArawT, aff matmul, lf2 LN
                f1T = [spool.tile([128, TN], bf16, tag=f"f1T{cc}", name=f"f1T{cc}")
                       for cc in range(2)]
                for cc in range(2):
                    pf = psA()
                    for h in range(8):
                        nc.tensor.matmul(
                            pf[:, h * 64 : h * 64 + 64],
                            gfr[h][:, cc * 128 : cc * 128 + 128],
                            atp[:, h * 64 : h * 64 + 64],
                            start=True, stop=True)
                    (nc.scalar.copy if cc == 0 else nc.vector.tensor_copy)(f1T[cc][:], pf[:])
                ArawT = spool.tile([BG, TN], bf16, tag="arawT", name="arawT")
                pat = psC()
                for ic in range(4):
                    nc.tensor.transpose(
                        pat[0:BG, ic * 128 : ic * 128 + 128], Araw[ic][:], ident[:])
                nc.vector.tensor_copy(ArawT[:], pat[0:BG, :])

                lf2 = [spool.tile([128, C], bf16, tag=f"lf2_{ic}", name=f"lf2_{ic}")
                       for ic in range(4)]
                lf2pre = [spool.tile([128, C], bf16, tag=f"lf2p{ic}", name=f"lf2p{ic}")
                          for ic in range(4)]
                st2w = spool.tile([128, 24], f32, tag="st2w", name="st2w")
                for ic in range(4):
                    pl = psB()
                    nc.tensor.matmul(
                        pl[:, 0:C], f1T[0][:, ic * 128 : ic * 128 + 128], wafft[0][:],
                        start=True, stop=False)
                    nc.tensor.matmul(
                        pl[:, 0:C], f1T[1][:, ic * 128 : ic * 128 + 128], wafft[1][:],
                        start=False, stop=False)
                    nc.tensor.matmul(
                        pl[:, 0:C], ArawT[0:BG, ic * 128 : ic * 128 + 128],
                        wafft[2][0:BG, :], start=False, stop=False)
                    nc.tensor.matmul(
                        pl[:, 0:C], onesb[0:1, :], baff_b[:],
                        start=False, stop=True)
                    nc.vector.bn_stats(st2w[:, ic * 6 : ic * 6 + 6], pl[:, 0:C])
                    nc.scalar.copy(lf2pre[ic][:], pl[:, 0:C])
                st24 = st2w[:].rearrange("p (i g v) -> p i g v", i=4, g=2)
                mu2 = spool.tile([128, 4], f32, tag="mu2", name="mu2")
                nc.vector.tensor_reduce(
                    mu2[:], st24[:, :, :, 1], axis=AX.X, op=OP.add)
                nc.vector.tensor_scalar_mul(mu2[:], mu2[:], 0.5)
                msq2 = spool.tile([128, 8], f32, tag="msq2", name="msq2")
                msq23 = msq2[:].rearrange("p (i g) -> p i g", i=4)
                nc.vector.tensor_tensor(
                    msq23, st24[:, :, :, 1], st24[:, :, :, 1], op=OP.mult)
                e2t = spool.tile([128, 8], f32, tag="e2t", name="e2t")
                nc.vector.scalar_tensor_tensor(
                    e2t[:].rearrange("p (i g) -> p i g", i=4),
                    st24[:, :, :, 2], 1.0 / 128.0, msq23, op0=OP.mult, op1=OP.add)
                ex2 = spool.tile([128, 4], f32, tag="ex2b", name="ex2b")
                nc.vector.tensor_reduce(
                    ex2[:], e2t[:].rearrange("p (i g) -> p i g", i=4),
                    axis=AX.X, op=OP.add)
                nc.vector.tensor_scalar_mul(ex2[:], ex2[:], 0.5)
                mu2sq = spool.tile([128, 4], f32, tag="mu2sq", name="mu2sq")
                nc.vector.tensor_tensor(mu2sq[:], mu2[:], mu2[:], op=OP.mult)
                nc.vector.tensor_tensor(ex2[:], ex2[:], mu2sq[:], op=OP.subtract)
                nc.vector.tensor_scalar_add(ex2[:], ex2[:], 1e-5)
                sd2 = spool.tile([128, 4], f32, tag="sd2", name="sd2")
                nc.scalar.sqrt(sd2[:], ex2[:])
                al2 = spool.tile([128, 4], f32, tag="al2", name="al2")
                nc.vector.reciprocal(al2[:], sd2[:])
                be2 = spool.tile([128, 4], f32, tag="be2", name="be2")
                nc.vector.scalar_tensor_tensor(
                    be2[:], mu2[:], -1.0, al2[:], op0=OP.mult, op1=OP.mult)
                for ic in range(4):
                    if trivial_gb2 and use_lrelu_act:
                        nc.scalar.activation(
                            lf2[ic][:], lf2pre[ic][:], AF.Lrelu,
                            bias=be2[:, ic : ic + 1], scale=al2[:, ic : ic + 1], alpha=0.01)
                    else:
                        nc.scalar.activation(
                            lf2[ic][:], lf2pre[ic][:], AF.Identity,
                            bias=be2[:, ic : ic + 1], scale=al2[:, ic : ic + 1])
                        if not trivial_gb2:
                            nc.vector.tensor_tensor(lf2[ic][:], lf2[ic][:], g2r[:], op=OP.mult)
                            nc.vector.tensor_tensor(lf2[ic][:], lf2[ic][:], b2r[:], op=OP.add)
                        nc.vector.scalar_tensor_tensor(
                            lf2[ic][:], lf2[ic][:], 0.01, lf2[ic][:],
                            op0=OP.mult, op1=OP.max)

                if stage <= 7:
                    for ic in range(4):
                        scc = spool.tile([128, C], f32, tag="stgc", name="stgc")
                        nc.scalar.copy(scc[:], lf2[ic][:])
                        dma(outb[ic * 128 : ic * 128 + 128, :], scc[:])
                    continue

                # S10: reduce
                catT = [spool.tile([128, TN], bf16, tag=f"catT{j}", name=f"catT{j}")
                        for j in range(4)]
                for cc in range(2):
                    ptx = psC(bf16)
                    for ic in range(4):
                        nc.tensor.transpose(
                            ptx[:, ic * 128 : ic * 128 + 128],
                            lf1b[ic][:, cc * 128 : cc * 128 + 128], identb[:])
                    (nc.scalar.copy if cc == 0 else nc.vector.tensor_copy)(catT[cc][:], ptx[:])
                    ptx2 = psC(bf16)
                    for ic in range(4):
                        nc.tensor.transpose(
                            ptx2[:, ic * 128 : ic * 128 + 128],
                            lf2[ic][:, cc * 128 : cc * 128 + 128], identb[:])
                    (nc.vector.tensor_copy if cc == 0 else nc.scalar.copy)(catT[2 + cc][:], ptx2[:])
                for ic in range(4):
                    pr = psA()
                    for j in range(4):
                        nc.tensor.matmul(
                            pr[:, 0:C], catT[j][:, ic * 128 : ic * 128 + 128], wredt[j][:],
                            start=(j == 0), stop=False)
                    nc.tensor.matmul(
                        pr[:, 0:C], onesb[0:1, :], bred_b[:],
                        start=False, stop=True)
                    if use_lrelu_act:
                        nc.scalar.activation(
                            red_sb[ic][:, b * C : (b + 1) * C], pr[:, 0:C],
                            AF.Lrelu, alpha=0.01)
                    else:
                        nc.scalar.copy(red_sb[ic][:, b * C : (b + 1) * C], pr[:, 0:C])
                        nc.vector.scalar_tensor_tensor(
                            red_sb[ic][:, b * C : (b + 1) * C],
                            red_sb[ic][:, b * C : (b + 1) * C], 0.01,
                            red_sb[ic][:, b * C : (b + 1) * C], op0=OP.mult, op1=OP.max)

                if stage <= 8:
                    for ic in range(4):
                        scc = spool.tile([128, C], f32, tag="stgc", name="stgc")
                        nc.scalar.copy(scc[:], red_sb[ic][:, b * C : (b + 1) * C])
                        dma(outb[ic * 128 : ic * 128 + 128, :], scc[:])
                    continue

                # S11: gate pieces (sigmoid deferred to end)
                str_ = spool.tile([128, 24], f32, tag="strn", name="strn")
                for ic in range(4):
                    nc.vector.bn_stats(
                        str_[:, ic * 6 : ic * 6 + 6], red_sb[ic][:, b * C : (b + 1) * C])
                str4 = str_[:].rearrange("p (i g v) -> p i g v", i=4, g=2)
                msqr = spool.tile([128, 8], f32, tag="msqr", name="msqr")
                msqr3 = msqr[:].rearrange("p (i g) -> p i g", i=4)
                nc.vector.tensor_tensor(
                    msqr3, str4[:, :, :, 1], str4[:, :, :, 1], op=OP.mult)
                m2sr = spool.tile([128, 8], f32, tag="m2sr", name="m2sr")
                nc.vector.scalar_tensor_tensor(
                    m2sr[:].rearrange("p (i g) -> p i g", i=4),
                    msqr3, 128.0, str4[:, :, :, 2], op0=OP.mult, op1=OP.add)
                ssqr = spool.tile([128, 4], f32, tag="ssqr", name="ssqr")
                nc.vector.tensor_reduce(
                    ssqr[:], m2sr[:].rearrange("p (i g) -> p i g", i=4),
                    axis=AX.X, op=OP.add)
                nrmr = spool.tile([128, 4], f32, tag="nrmr", name="nrmr")
                nc.scalar.sqrt(nrmr[:], ssqr[:])
                nc.vector.reciprocal(rinv_w[:, b * 4 : b * 4 + 4], nrmr[:])
                pv = psA()
                for ic in range(4):
                    wa2 = spool.tile([128, 1], bf16, tag="wa2", name="wa2")
                    nc.vector.tensor_tensor(
                        wa2[:], wa_col[:, ic : ic + 1],
                        rinv_w[:, b * 4 + ic : b * 4 + ic + 1], op=OP.mult)
                    nc.tensor.matmul(
                        pv[0:1, 0:C], wa2[:], red_sb[ic][:, b * C : (b + 1) * C],
                        start=(ic == 0), stop=(ic == 3))
                vrow = spool.tile([1, C], bf16, tag="vrow", name="vrow")
                nc.scalar.copy(vrow[:], pv[0:1, 0:C])
                pvr = psA()
                nc.tensor.matmul(pvr[0:128, 0:C], onesb[:], vrow[:], start=True, stop=True)
                vrep = spool.tile([128, C], bf16, tag="vrep", name="vrep")
                nc.vector.tensor_copy(vrep[:], pvr[0:128, 0:C])
                s0w = spool.tile([128, 4], f32, tag="s0w", name="s0w")
                pw0 = spool.tile([128, 4], f32, tag="pw0", name="pw0")
                for ic in range(4):
                    scr = spool.tile([128, C], bf16, tag="gscr", name="gscr")
                    nc.vector.scalar_tensor_tensor(
                        scr[:], red_sb[ic][:, b * C : (b + 1) * C], 1.0, vrep[:],
                        op0=OP.mult, op1=OP.mult,
                        accum_out=s0w[:, ic : ic + 1])
                    scr3 = spool.tile([128, POSD], f32, tag="gscr3", name="gscr3")
                    nc.vector.scalar_tensor_tensor(
                        scr3[:], pos_w[ic][:, b * POSD : (b + 1) * POSD], 1.0, wp_rep[:],
                        op0=OP.mult, op1=OP.mult,
                        accum_out=pw0[:, ic : ic + 1])
                gtmp = spool.tile([128, 4], f32, tag="gtmp", name="gtmp")
                nc.vector.tensor_tensor(
                    gtmp[:], s0w[:], rinv_w[:, b * 4 : b * 4 + 4], op=OP.mult)
                nc.vector.tensor_tensor(gtmp[:], gtmp[:], pw0[:], op=OP.add)
                nc.vector.tensor_tensor(
                    garg_w[:, b * 4 : b * 4 + 4], gtmp[:],
                    batt_rep[:].broadcast_to([128, 4]), op=OP.add)

            # ---------------- end: sigmoid + output ----------------
            if stage > 8:
                att_w16 = wpool.tile([128, 4 * bpc], f32)
                nc.scalar.activation(att_w16[:], garg_w[:], AF.Sigmoid)
                for b in range(bpc):
                    outb = out_d[b].flatten_outer_dims()
                    for ic in range(4):
                        outsb = spool.tile([128, C], f32, tag="outsb", name="outsb")
                        nc.vector.tensor_scalar_mul(
                            outsb[:], red_sb[ic][:, b * C : (b + 1) * C],
                            att_w16[:, b * 4 + ic : b * 4 + ic + 1])
                        dma(outb[ic * 128 : ic * 128 + 128, :], outsb[:])

    nc.finalize()
    return nc


_CACHE = {}


def _get_nc(bpc, trivial_gb1, trivial_gb2, use_lrelu_act=True, stage=9):
    key = (bpc, trivial_gb1, trivial_gb2, use_lrelu_act, stage)
    if key not in _CACHE:
        _CACHE[key] = build_nc(*key)
    return _CACHE[key]


def make_in_maps(inputs, ncores):
    lf = np.asarray(inputs["local_feat"], np.float32)
    gf = np.asarray(inputs["global_feat"], np.float32)
    pos = np.asarray(inputs["pos"], np.float32)
    bpc = lf.shape[0] // ncores

    wcv = np.asarray(inputs["tc_conv_w"], np.float32).reshape(C, KK)
    # Wshift tiles: [idx][row, half*C + c] = wcv[c, 4*jc-2*p-half + row//32] (0 if k out of range)
    wshift = np.zeros((NPAIR, 128, 2 * C), np.float32)
    for (jc, p), i in PAIR_IDX.items():
        for tl in range(4):
            for half in range(2):
                k = 4 * jc - (2 * p + half) + tl
                if 0 <= k < KK:
                    wshift[i, 32 * tl : 32 * tl + 32, half * C : (half + 1) * C] = wcv[:, k][None, :]
    waff = np.asarray(inputs["bi_aff_w"], np.float32)  # (C, C+BG)
    wafft = np.zeros((3, 128, C), np.float32)
    for j in range(3):
        kdim = 128 if j < 2 else BG
        wafft[j, :kdim, :] = waff[:, j * 128 : j * 128 + kdim].T
    redw = np.asarray(inputs["red_w"], np.float32)  # (C, 2C)
    wredt = np.zeros((4, 128, C), np.float32)
    for j in range(4):
        wredt[j] = redw[:, j * 128 : j * 128 + 128].T
    attw = np.asarray(inputs["att_w"], np.float32).reshape(-1)
    wa_col = np.ascontiguousarray(attw[:TN].reshape(4, 128).T)  # (128, 4)
    wp_rep = np.tile(attw[TN : TN + POSD][None, :], (128, 1))
    batt_rep = np.full((128, 1), float(np.asarray(inputs["att_b"]).reshape(-1)[0]), np.float32)
    g1r = np.tile(np.asarray(inputs["tc_ln_g"], np.float32).reshape(1, C), (128, 1))
    b1r = np.tile(np.asarray(inputs["tc_ln_b"], np.float32).reshape(1, C), (128, 1))
    g2r = np.tile(np.asarray(inputs["bi_ln_g"], np.float32).reshape(1, C), (128, 1))
    b2r = np.tile(np.asarray(inputs["bi_ln_b"], np.float32).reshape(1, C), (128, 1))
    ident = np.eye(128, dtype=np.float32)

    params = {
        "tc_adj_w": np.ascontiguousarray(np.asarray(inputs["tc_adj_w"], np.float32)),
        "bi_adj_w": np.ascontiguousarray(np.asarray(inputs["bi_adj_w"], np.float32)),
        "wshift": wshift,
        "wafft": wafft,
        "bi_aff_b": np.asarray(inputs["bi_aff_b"], np.float32).reshape(1, C),
        "wredt": wredt,
        "red_b": np.asarray(inputs["red_b"], np.float32).reshape(1, C),
        "ident": ident,
        "identb": ident,
        "wa_col": wa_col,
        "wp_rep": wp_rep,
        "batt_rep": batt_rep,
        "g1r": g1r, "b1r": b1r, "g2r": g2r, "b2r": b2r,
    }
    in_maps = []
    for core in range(ncores):
        sl = slice(core * bpc, (core + 1) * bpc)
        m = dict(params)
        m["local_feat"] = np.ascontiguousarray(lf[sl])
        m["global_feat"] = np.ascontiguousarray(gf[sl])
        m["pos"] = np.ascontiguousarray(pos[sl])
        in_maps.append(m)
    return in_maps, bpc


def kernel(**inputs):
    from concourse.bass_utils import run_bass_kernel_spmd

    trivial_gb1 = bool(
        np.allclose(inputs["tc_ln_g"], 1.0) and np.allclose(inputs["tc_ln_b"], 0.0)
    )
    trivial_gb2 = bool(
        np.allclose(inputs["bi_ln_g"], 1.0) and np.allclose(inputs["bi_ln_b"], 0.0)
    )
    in_maps, bpc = make_in_maps(inputs, NCORES)
    nc = _get_nc(bpc, trivial_gb1, trivial_gb2)
    res = run_bass_kernel_spmd(nc, in_maps, core_ids=list(range(NCORES)))
    outs = [res.results[c]["out"] for c in range(NCORES)]
    return np.concatenate(outs, axis=0).reshape(B, T, N, C)


if __name__ == "__main__":
    nc = build_nc(1, True, True)
    print("build ok")


# revision 23
# speedup vs baseline: 1.1728x; 1.1728x over previous
"""Trainium2 Bass kernel for nn_Comb5 (gnn_message_passing) — v2.

Data-parallel over batch: 32 batches -> 8 cores x 4 batches.
Key structure vs v1:
 - conv tap tiles (ED2) built by one DVE multiply against host-precomputed
   shifted-weight tiles (Wshift) instead of 56 PE matmuls + 50 narrow copies.
 - A_raw computed block-diagonally (per-t 32x49 matmuls, col-group packed).
 - feat1^T computed directly on PE from a compact block-diag Asm^T (ATP),
   no gpsimd DMAs / padded AsmPT.
 - branch1 LN row-sums fused into the PSUM->SBUF drain on ScalarE
   (activation Identity + accum_out); sumsq via tensor_tensor_reduce bf16.
 - rownorms via bn_stats + small wide combines; one Sqrt site per use.
 - single Sigmoid at end of program; weights pre-transposed on host.
"""

import sys

sys.path.insert(0, "/opt/trn_rl_repo")

import numpy as np
import ml_dtypes

from concourse import bass, bacc, tile, mybir

f32 = mybir.dt.float32
f32r = mybir.dt.float32r
bf16 = mybir.dt.bfloat16
AX = mybir.AxisListType
OP = mybir.AluOpType
AF = mybir.ActivationFunctionType

B, T, N, C, BG, POSD, KK = 32, 16, 32, 256, 49, 9, 7
TN = T * N          # 512
OT = T - KK + 1     # 10
NCORES = 8


def jcs_of(ot):
    return list(range(ot // 4, min(3, (ot + 6) // 4) + 1))


# (jc, p) pairs for the pair-packed conv tap tiles
PAIRS = []
for p in range(OT // 2):
    for jc in sorted(set(jcs_of(2 * p)) | set(jcs_of(2 * p + 1))):
        PAIRS.append((jc, p))
PAIR_IDX = {k: i for i, k in enumerate(PAIRS)}
NPAIR = len(PAIRS)  # 12

# conv matmul groups: per och chunk, list of (p, [jc...])
OCH_GROUPS = []
for och in range(3):
    prs = list(range(och * 2, min(OT // 2, och * 2 + 2)))
    OCH_GROUPS.append([(p, sorted(set(jcs_of(2 * p)) | set(jcs_of(2 * p + 1))))
                       for p in prs])


def _r(ap):
    return ap.bitcast(f32r)


def build_nc(bpc, trivial_gb1, trivial_gb2, use_lrelu_act=True, stage=9):
    nc = bacc.Bacc(target_bir_lowering=False, debug=False)

    lf_d = nc.declare_dram_parameter("local_feat", [bpc, T, N, C], f32, isOutput=False)
    gf_d = nc.declare_dram_parameter("global_feat", [bpc, T, BG, C], f32, isOutput=False)
    pos_d = nc.declare_dram_parameter("pos", [bpc, T, N, POSD], f32, isOutput=False)
    w1_d = nc.declare_dram_parameter("tc_adj_w", [C, C], f32, isOutput=False)
    w2_d = nc.declare_dram_parameter("bi_adj_w", [C, C], f32, isOutput=False)
    wsh_d = nc.declare_dram_parameter("wshift", [NPAIR, 128, 2 * C], f32, isOutput=False)
    wafft_d = nc.declare_dram_parameter("wafft", [3, 128, C], f32, isOutput=False)
    baff_d = nc.declare_dram_parameter("bi_aff_b", [1, C], f32, isOutput=False)
    wredt_d = nc.declare_dram_parameter("wredt", [4, 128, C], f32, isOutput=False)
    bred_d = nc.declare_dram_parameter("red_b", [1, C], f32, isOutput=False)
    ident_d = nc.declare_dram_parameter("ident", [128, 128], f32, isOutput=False)
    identb_d = nc.declare_dram_parameter("identb", [128, 128], f32, isOutput=False)
    wacol_d = nc.declare_dram_parameter("wa_col", [128, 4], f32, isOutput=False)
    wprep_d = nc.declare_dram_parameter("wp_rep", [128, POSD], f32, isOutput=False)
    batt_d = nc.declare_dram_parameter("batt_rep", [128, 1], f32, isOutput=False)
    g1_d = nc.declare_dram_parameter("g1r", [128, C], f32, isOutput=False)
    b1_d = nc.declare_dram_parameter("b1r", [128, C], f32, isOutput=False)
    g2_d = nc.declare_dram_parameter("g2r", [128, C], f32, isOutput=False)
    b2_d = nc.declare_dram_parameter("b2r", [128, C], f32, isOutput=False)
    out_d = nc.declare_dram_parameter("out", [bpc, T, N, C], f32, isOutput=True)

    dma = nc.sync.dma_start

    with tile.TileContext(nc) as tc:
        with (
            tc.tile_pool(name="const", bufs=1) as cpool,
            tc.tile_pool(name="work", bufs=1) as wpool,
            tc.tile_pool(name="scr", bufs=2) as spool,
            tc.tile_pool(name="ps", bufs=1, space="PSUM") as psp,
        ):
            def psA(dt=f32):
                return psp.tile([128, 512], dt, tag="pA", name="pA", bufs=2)

            def psB(dt=f32):
                return psp.tile([128, 1024], dt, tag="pB", name="pB", bufs=2)

            def psC(dt=f32):
                return psp.tile([128, 512], dt, tag="pC", name="pC", bufs=2)

            # ---------------- constants (pure DMA) ----------------
            ident = cpool.tile([128, 128], f32)
            dma(ident[:], ident_d[:])
            identb = cpool.tile([128, 128], bf16)
            idld = spool.tile([128, 128], f32, tag="idld", name="idld")
            dma(idld[:], identb_d[:])
            nc.scalar.copy(identb[:], idld[:])
            wsh = [cpool.tile([128, 2 * C], bf16, tag=f"wsh{i}", name=f"wsh{i}")
                   for i in range(NPAIR)]
            for i in range(NPAIR):
                wshld = spool.tile([128, 2 * C], f32, tag="wshld", name="wshld")
                dma(wshld[:], wsh_d[i])
                (nc.scalar.copy if i % 2 else nc.vector.tensor_copy)(wsh[i][:], wshld[:])
            w1 = [cpool.tile([128, C], f32, tag=f"w1_{kc}", name=f"w1_{kc}") for kc in range(2)]
            w2 = [cpool.tile([128, C], f32, tag=f"w2_{kc}", name=f"w2_{kc}") for kc in range(2)]
            for kc in range(2):
                wld = spool.tile([128, C], f32, tag="wld", name="wld")
                dma(wld[:], w1_d[kc * 128 : kc * 128 + 128, :])
                nc.scalar.copy(w1[kc][:].bitcast(f32r), wld[:])
                wld2 = spool.tile([128, C], f32, tag="wld2", name="wld2")
                dma(wld2[:], w2_d[kc * 128 : kc * 128 + 128, :])
                nc.scalar.copy(w2[kc][:].bitcast(f32r), wld2[:])
            wafft = [cpool.tile([128, C], bf16, tag=f"wafft{j}", name=f"wafft{j}") for j in range(3)]
            for j in range(3):
                wald = spool.tile([128, C], f32, tag="wald", name="wald")
                dma(wald[:], wafft_d[j])
                nc.vector.tensor_copy(wafft[j][:], wald[:])
            wredt = [cpool.tile([128, C], bf16, tag=f"wredt{j}", name=f"wredt{j}") for j in range(4)]
            for j in range(4):
                wrld = spool.tile([128, C], f32, tag="wrld", name="wrld")
                dma(wrld[:], wredt_d[j])
                nc.scalar.copy(wredt[j][:], wrld[:])
            wa_col = cpool.tile([128, 4], f32)
            dma(wa_col[:], wacol_d[:])
            wp_rep = cpool.tile([128, POSD], f32)
            dma(wp_rep[:], wprep_d[:])
            batt_rep = cpool.tile([128, 1], f32)
            dma(batt_rep[:], batt_d[:])
            baff_row = cpool.tile([1, C], f32)
            dma(baff_row[:], baff_d[:])
            bred_row = cpool.tile([1, C], f32)
            dma(bred_row[:], bred_d[:])
            onesb = cpool.tile([1, 128], bf16)
            nc.vector.memset(onesb[:], 1.0)
            g1r = b1r = g2r = b2r = None
            if not trivial_gb1:
                g1r = cpool.tile([128, C], f32, tag="g1r", name="g1r")
                b1r = cpool.tile([128, C], f32, tag="b1r", name="b1r")
                dma(g1r[:], g1_d[:])
                dma(b1r[:], b1_d[:])
            if not trivial_gb2:
                g2r = cpool.tile([128, C], f32, tag="g2r", name="g2r")
                b2r = cpool.tile([128, C], f32, tag="b2r", name="b2r")
                dma(g2r[:], g2_d[:])
                dma(b2r[:], b2_d[:])
            # aff / red bias rows in bf16 (added via K=1 matmul into accum groups)
            baff_b = cpool.tile([1, C], bf16)
            bred_b = cpool.tile([1, C], bf16)
            nc.vector.tensor_copy(baff_b[:], baff_row[:])
            nc.vector.tensor_copy(bred_b[:], bred_row[:])

            # persistent
            pos_w = [wpool.tile([128, bpc * POSD], f32, tag=f"pos{i}", name=f"pos{i}")
                     for i in range(4)]
            red_sb = [wpool.tile([128, bpc * C], bf16, tag=f"red{i}", name=f"red{i}")
                      for i in range(4)]
            garg_w = wpool.tile([128, 4 * bpc], f32)
            rinv_w = wpool.tile([128, 4 * bpc], f32)

            # ATP: persistent, zero pads written once
            atp = wpool.tile([128, 512], bf16, tag="ATP", name="ATP")
            nc.vector.memset(atp[:], 0.0)
            # gf tiles persistent: pad rows (49:64, 113:128) must stay finite
            gf = [wpool.tile([128, C], f32, tag=f"gf{h}", name=f"gf{h}") for h in range(8)]
            for h in range(8):
                nc.vector.memset(gf[h][:], 1.0)

            # ---------------- per batch ----------------
            for b in range(bpc):
                lfb_dram = lf_d[b].flatten_outer_dims()
                gfb_dram = gf_d[b].flatten_outer_dims()
                posb_dram = pos_d[b].flatten_outer_dims()
                outb = out_d[b].flatten_outer_dims()

                # S1: loads
                lf = [spool.tile([128, C], f32, tag=f"lf{i}", name=f"lf{i}") for i in range(4)]
                for ic in range(4):
                    dma(lf[ic][:], lfb_dram[ic * 128 : ic * 128 + 128, :])
                for t in range(T):
                    dma(gf[t // 2][64 * (t % 2) : 64 * (t % 2) + BG, :],
                        gfb_dram[t * BG : t * BG + BG, :])
                for ic in range(4):
                    dma(pos_w[ic][:, b * POSD : (b + 1) * POSD],
                        posb_dram[ic * 128 : ic * 128 + 128, :])

                # S2: lf rownorm -> nf, lfb
                # bn_stats layout per op: [cnt1, m1, M2_1, cnt2, m2, M2_2]
                # ssq = M2a+M2b + 128*(ma^2+mb^2)
                stl = spool.tile([128, 24], f32, tag="stl", name="stl")
                for ic in range(4):
                    nc.vector.bn_stats(stl[:, ic * 6 : ic * 6 + 6], lf[ic][:])
                stl4 = stl[:].rearrange("p (i g v) -> p i g v", i=4, g=2)  # v=3
                msq = spool.tile([128, 8], f32, tag="msq", name="msq")
                msq3 = msq[:].rearrange("p (i g) -> p i g", i=4)
                nc.vector.tensor_tensor(
                    msq3, stl4[:, :, :, 1], stl4[:, :, :, 1], op=OP.mult)
                m2s = spool.tile([128, 8], f32, tag="m2s", name="m2s")
                nc.vector.scalar_tensor_tensor(
                    m2s[:].rearrange("p (i g) -> p i g", i=4),
                    msq3, 128.0, stl4[:, :, :, 2], op0=OP.mult, op1=OP.add)
                ssqA = spool.tile([128, 12], f32, tag="ssqA", name="ssqA")
                nc.vector.tensor_reduce(
                    ssqA[:, 0:4], m2s[:].rearrange("p (i g) -> p i g", i=4),
                    axis=AX.X, op=OP.add)
                # S3: gf rownorm -> nfg (pads garbage, never read)
                stg = spool.tile([128, 48], f32, tag="stg", name="stg")
                for h in range(8):
                    nc.vector.bn_stats(stg[:, h * 6 : h * 6 + 6], gf[h][:])
                stg4 = stg[:].rearrange("p (i g v) -> p i g v", i=8, g=2)
                msqg = spool.tile([128, 16], f32, tag="msqg", name="msqg")
                msqg3 = msqg[:].rearrange("p (i g) -> p i g", i=8)
                nc.vector.tensor_tensor(
                    msqg3, stg4[:, :, :, 1], stg4[:, :, :, 1], op=OP.mult)
                m2sg = spool.tile([128, 16], f32, tag="m2sg", name="m2sg")
                nc.vector.scalar_tensor_tensor(
                    m2sg[:].rearrange("p (i g) -> p i g", i=8),
                    msqg3, 128.0, stg4[:, :, :, 2], op0=OP.mult, op1=OP.add)
                nc.vector.tensor_reduce(
                    ssqA[:, 4:12], m2sg[:].rearrange("p (i g) -> p i g", i=8),
                    axis=AX.X, op=OP.add)
                nrmA = spool.tile([128, 12], f32, tag="nrmA", name="nrmA")
                nc.scalar.sqrt(nrmA[:], ssqA[:])
                invA = spool.tile([128, 12], f32, tag="invA", name="invA")
                nc.vector.reciprocal(invA[:], nrmA[:])
                inv4 = invA[:, 0:4]
                inv8 = invA[:, 4:12]
                nf = [spool.tile([128, C], f32, tag=f"nf{i}", name=f"nf{i}", bufs=1) for i in range(4)]
                lfb = [spool.tile([128, C], bf16, tag=f"lfb{i}", name=f"lfb{i}") for i in range(4)]
                for ic in range(4):
                    nc.vector.tensor_scalar_mul(nf[ic][:], lf[ic][:], inv4[:, ic : ic + 1])
                    nc.scalar.copy(lfb[ic][:], lf[ic][:])
                nfg = [spool.tile([128, C], bf16, tag=f"nfg{h}", name=f"nfg{h}", bufs=1) for h in range(8)]
                gfr = [spool.tile([128, C], bf16, tag=f"gfr{h}", name=f"gfr{h}") for h in range(8)]
                for h in range(8):
                    nc.vector.tensor_scalar_mul(nfg[h][:], gf[h][:], inv8[:, h : h + 1])
                    nc.scalar.copy(gfr[h][:], gf[h][:])

                # S4: transposes
                nfT = [spool.tile([128, TN], f32, tag=f"nfT{cc}", name=f"nfT{cc}") for cc in range(2)]
                for cc in range(2):
                    pt = psC()
                    for jc in range(4):
                        nc.tensor.transpose(
                            pt[:, jc * 128 : jc * 128 + 128],
                            nf[jc][:, cc * 128 : cc * 128 + 128], ident[:])
                    (nc.scalar.copy if cc == 0 else nc.vector.tensor_copy)(
                        nfT[cc][:].bitcast(f32r), pt[:])
                nfgT = [spool.tile([128, 1024], bf16, tag=f"nfgT{cc}", name=f"nfgT{cc}", bufs=1)
                        for cc in range(2)]
                for cc in range(2):
                    pt = psB(bf16)
                    for h in range(8):
                        nc.tensor.transpose(
                            pt[:, h * 128 : h * 128 + 128],
                            nfg[h][:, cc * 128 : cc * 128 + 128], identb[:])
                    (nc.scalar.copy if cc == 0 else nc.vector.tensor_copy)(
                        nfgT[cc][:], pt[:])

                if stage <= 2:
                    for ic in range(4):
                        dma(outb[ic * 128 : ic * 128 + 128, :], nf[ic][:])
                    continue

                # S5: branch1 A chain -> Ind -> IndT
                ut = [spool.tile([128, TN], f32, tag=f"ut{cc}", name=f"ut{cc}") for cc in range(2)]
                for cc in range(2):
                    pu = psA()
                    for kc in range(2):
                        nc.tensor.matmul(
                            pu[:], _r(w1[kc][:, cc * 128 : cc * 128 + 128]), _r(nfT[kc][:]),
                            start=(kc == 0), stop=(kc == 1))
                    (nc.scalar.copy if cc == 0 else nc.vector.tensor_copy)(
                        ut[cc][:].bitcast(f32r), pu[:])
                Ind = [spool.tile([128, TN], bf16, tag=f"ind{ic}", name=f"ind{ic}") for ic in range(4)]
                for ic in range(4):
                    pa = psA()
                    for kc in range(2):
                        nc.tensor.matmul(
                            pa[:], _r(ut[kc][:, ic * 128 : ic * 128 + 128]), _r(nfT[kc][:]),
                            start=(kc == 0), stop=(kc == 1))
                    bmax = spool.tile([128, T], f32, tag="bmax", name="bmax")
                    nc.vector.tensor_reduce(
                        bmax[:], pa[:].rearrange("p (t n) -> p t n", t=T),
                        axis=AX.X, op=OP.max)
                    nc.vector.tensor_tensor(
                        Ind[ic][:].rearrange("p (t n) -> p t n", t=T),
                        pa[:].rearrange("p (t n) -> p t n", t=T),
                        bmax[:].unsqueeze(2).broadcast_to([128, T, N]),
                        op=OP.is_equal)

                if stage <= 3:
                    for ic in range(4):
                        scc = spool.tile([128, C], f32, tag="stgc", name="stgc")
                        nc.scalar.copy(scc[:], Ind[ic][:, 0:C])
                        dma(outb[ic * 128 : ic * 128 + 128, :], scc[:])
                    continue

                IndT = [spool.tile([128, TN], bf16, tag=f"indT{jc}", name=f"indT{jc}")
                        for jc in range(4)]
                for jc in range(4):
                    pt = psC(bf16)
                    for ic in range(4):
                        nc.tensor.transpose(
                            pt[:, ic * 128 : ic * 128 + 128],
                            Ind[ic][:, jc * 128 : jc * 128 + 128], identb[:])
                    (nc.vector.tensor_copy if jc % 2 else nc.scalar.copy)(IndT[jc][:], pt[:])

                # S5.5: ED2 tiles via one DVE mult each
                ED2 = {}
                for (jc, p) in PAIRS:
                    i = PAIR_IDX[(jc, p)]
                    tl_ = spool.tile([128, 2 * C], bf16, tag=f"ed2_{i}", name=f"ed2_{i}", bufs=1)
                    nc.vector.tensor_tensor(
                        tl_[:].rearrange("p (h c) -> p h c", h=2),
                        lfb[jc][:].unsqueeze(1).broadcast_to([128, 2, C]),
                        wsh[i][:].rearrange("p (h c) -> p h c", h=2),
                        op=OP.mult)
                    ED2[(jc, p)] = tl_

                # S6: conv -> y (+s1 on drain), s2, LN combine, z (lrelu), lf1
                s1w = spool.tile([128, 40], f32, tag="s1w", name="s1w")
                s2w = spool.tile([128, 40], f32, tag="s2w", name="s2w")
                y_sb = [spool.tile([128, OT * C], bf16, tag=f"y{ic}", name=f"y{ic}", bufs=1)
                        for ic in range(4)]
                for ic in range(4):
                    for och in range(3):
                        # jc-outer so the IndT stationary is reused across p's
                        groups = OCH_GROUPS[och]
                        seq = []
                        for pi, (p, pjcs) in enumerate(groups):
                            for ji, jc in enumerate(pjcs):
                                seq.append((jc, pi, p, ji == 0, ji == len(pjcs) - 1))
                        seq.sort(key=lambda x: x[0])
                        py = psB()
                        for jc, pi, p, st_, sp_ in seq:
                            nc.tensor.matmul(
                                py[:, pi * 2 * C : pi * 2 * C + 2 * C],
                                IndT[jc][:, ic * 128 : ic * 128 + 128],
                                ED2[(jc, p)][:],
                                start=st_, stop=sp_)
                        n_el = 2 * len(groups) * C
                        nc.scalar.copy(
                            y_sb[ic][:, och * 4 * C : och * 4 * C + n_el],
                            py[:, 0:n_el])
                    stb = spool.tile([128, 30], f32, tag=f"stb{ic}", name=f"stb{ic}")
                    for o5 in range(5):
                        nc.vector.bn_stats(
                            stb[:, o5 * 6 : o5 * 6 + 6],
                            y_sb[ic][:, o5 * 2 * C : (o5 + 1) * 2 * C])
                    stbv = stb[:].rearrange("p (o s v) -> p o s v", o=5, s=2)
                    nc.vector.tensor_copy(
                        s1w[:, ic * 10 : ic * 10 + 10].rearrange("p (o s) -> p o s", o=5),
                        stbv[:, :, :, 1])
                    nc.vector.tensor_copy(
                        s2w[:, ic * 10 : ic * 10 + 10].rearrange("p (o s) -> p o s", o=5),
                        stbv[:, :, :, 2])
                if stage == 4:
                    for ic in range(4):
                        scc4 = spool.tile([128, C], f32, tag="stgc", name="stgc")
                        nc.scalar.copy(scc4[:], y_sb[ic][:, 0:C])
                        dma(outb[ic * 128 : ic * 128 + 128, :], scc4[:])
                    continue
                muw = s1w
                exw = spool.tile([128, 40], f32, tag="exw", name="exw")
                nc.vector.tensor_scalar(
                    exw[:], s2w[:], 1.0 / 256.0, 1e-5, op0=OP.mult, op1=OP.add)
                sdw = spool.tile([128, 40], f32, tag="sdw", name="sdw")
                nc.scalar.sqrt(sdw[:], exw[:])
                alw = spool.tile([128, 40], f32, tag="alw", name="alw")
                nc.vector.reciprocal(alw[:], sdw[:])
                bew = spool.tile([128, 40], f32, tag="bew", name="bew")
                nc.vector.scalar_tensor_tensor(
                    bew[:], muw[:], -1.0, alw[:], op0=OP.mult, op1=OP.mult)
                lf1b = [spool.tile([128, C], bf16, tag=f"lf1b{ic}", name=f"lf1b{ic}")
                        for ic in range(4)]
                for ic in range(4):
                    for ot in range(OT):
                        sl = slice(ot * C, (ot + 1) * C)
                        col = slice(ic * 10 + ot, ic * 10 + ot + 1)
                        if trivial_gb1 and use_lrelu_act:
                            nc.scalar.activation(
                                y_sb[ic][:, sl], y_sb[ic][:, sl], AF.Lrelu,
                                bias=bew[:, col], scale=alw[:, col], alpha=0.01)
                        else:
                            nc.scalar.activation(
                                y_sb[ic][:, sl], y_sb[ic][:, sl], AF.Identity,
                                bias=bew[:, col], scale=alw[:, col])
                            if not trivial_gb1:
                                nc.vector.tensor_tensor(
                                    y_sb[ic][:, sl], y_sb[ic][:, sl], g1r[:], op=OP.mult)
                                nc.vector.tensor_tensor(
                                    y_sb[ic][:, sl], y_sb[ic][:, sl], b1r[:], op=OP.add)
                            nc.vector.scalar_tensor_tensor(
                                y_sb[ic][:, sl], y_sb[ic][:, sl], 0.01,
                                y_sb[ic][:, sl], op0=OP.mult, op1=OP.max)
                    # mean over ot: pairwise tree (z viewed (p, 5, 2, 256))
                    zv = y_sb[ic][:].rearrange("p (u e c) -> p u e c", u=5, e=2)
                    mtr = spool.tile([128, 5 * C], bf16, tag="mtr", name="mtr")
                    nc.vector.tensor_tensor(
                        mtr[:].rearrange("p (u c) -> p u c", u=5),
                        zv[:, :, 0], zv[:, :, 1], op=OP.add)
                    mv = mtr[:].rearrange("p (u c) -> p u c", u=5)
                    m2 = spool.tile([128, 2 * C], bf16, tag="mt2", name="mt2")
                    nc.vector.tensor_tensor(
                        m2[:].rearrange("p (u c) -> p u c", u=2),
                        mv[:, 0:2], mv[:, 2:4], op=OP.add)
                    m3 = spool.tile([128, C], bf16, tag="mt3", name="mt3")
                    nc.vector.tensor_tensor(
                        m3[:], m2[:, 0:C], m2[:, C : 2 * C], op=OP.add)
                    nc.vector.tensor_tensor(m3[:], m3[:], mv[:, 4], op=OP.add)
                    nc.vector.tensor_scalar_mul(lf1b[ic][:], m3[:], 1.0 / OT)

                if stage <= 5:
                    for ic in range(4):
                        scc = spool.tile([128, C], f32, tag="stgc", name="stgc")
                        nc.scalar.copy(scc[:], lf1b[ic][:])
                        dma(outb[ic * 128 : ic * 128 + 128, :], scc[:])
                    continue

                # S7: branch2 A chain (block-diag) -> softmax
                ut2 = [spool.tile([128, TN], bf16, tag=f"ut2{cc}", name=f"ut2{cc}") for cc in range(2)]
                for cc in range(2):
                    pu = psA()
                    for kc in range(2):
                        nc.tensor.matmul(
                            pu[:], _r(w2[kc][:, cc * 128 : cc * 128 + 128]), _r(nfT[kc][:]),
                            start=(kc == 0), stop=(kc == 1))
                    (nc.scalar.copy if cc == 0 else nc.vector.tensor_copy)(
                        ut2[cc][:], pu[:])
                Araw = [spool.tile([128, BG], f32, tag=f"araw{ic}", name=f"araw{ic}")
                        for ic in range(4)]
                Asm = [spool.tile([128, BG], bf16, tag=f"asm{ic}", name=f"asm{ic}")
                       for ic in range(4)]
                den_scr = spool.tile([128, BG], f32, tag="denscr", name="denscr")
                for ic in range(4):
                    pa = psA()
                    for tl in range(4):
                        t_g = ic * 4 + tl
                        for kc in range(2):
                            nc.tensor.matmul(
                                pa[32 * tl : 32 * tl + 32, 0:BG],
                                ut2[kc][:, t_g * 32 : t_g * 32 + 32],
                                nfgT[kc][:, t_g * 64 : t_g * 64 + BG],
                                start=(kc == 0), stop=(kc == 1),
                                tile_position=(0, 32 * tl))
                    nc.vector.tensor_copy(Araw[ic][:], pa[:, 0:BG])
                    rmax = spool.tile([128, 1], f32, tag="rmax", name="rmax")
                    nc.vector.tensor_reduce(rmax[:], Araw[ic][:], axis=AX.X, op=OP.max)
                    nbias = spool.tile([128, 1], f32, tag="nbias", name="nbias")
                    nc.vector.tensor_scalar_mul(nbias[:], rmax[:], -5.0)
                    den = spool.tile([128, 1], f32, tag="smden", name="smden")
                    nc.scalar.activation(
                        den_scr[:], Araw[ic][:], AF.Exp, bias=nbias[:], scale=5.0,
                        accum_out=den[:])
                    rden = spool.tile([128, 1], f32, tag="smrden", name="smrden")
                    nc.vector.reciprocal(rden[:], den[:])
                    nc.vector.tensor_scalar_mul(Asm[ic][:], den_scr[:], rden[:])

                # S8: ATP (block-diag Asm^T) via sub-transposes
                for ic in range(4):
                    ptb = psC(bf16)
                    for tl in range(4):
                        po = 64 * (tl % 2)
                        nc.tensor.transpose(
                            ptb[po : po + BG, 32 * tl : 32 * tl + 32],
                            Asm[ic][32 * tl : 32 * tl + 32, :],
                            identb[32 * tl : 32 * tl + 32, 32 * tl : 32 * tl + 32],
                            tile_position=(32 * tl, po))
                    # merged copies: even tl -> rows 0:49 (i-even slot), odd -> rows 64:113
                    # psum cols 32*tl -> (pair a, parity e, 32); atp cols h*64+32*par
                    pv4 = ptb[0:BG, 0:128].rearrange("p (a e c) -> p a e c", a=2, e=2)
                    av4 = atp[0:BG, 128 * ic : 128 * ic + 128].rearrange(
                        "p (h e c) -> p h e c", h=2, e=2)
                    nc.vector.tensor_copy(av4[:, :, 0], pv4[:, :, 0])
                    pv4o = ptb[64 : 64 + BG, 0:128].rearrange("p (a e c) -> p a e c", a=2, e=2)
                    av4o = atp[64 : 64 + BG, 128 * ic : 128 * ic + 128].rearrange(
                        "p (h e c) -> p h e c", h=2, e=2)
                    nc.vector.tensor_copy(av4o[:, :, 1], pv4o[:, :, 1])

                # S9: feat1^T, ArawT, aff matmul, lf2 LN
                f1T = [spool.tile([128, TN], bf16, tag=f"f1T{cc}", name=f"f1T{cc}")
                       for cc in range(2)]
                for cc in range(2):
                    pf = psA()
                    for h in range(8):
                        nc.tensor.matmul(
                            pf[:, h * 64 : h * 64 + 64],
                            gfr[h][:, cc * 128 : cc * 128 + 128],
                            atp[:, h * 64 : h * 64 + 64],
                            start=True, stop=True)
                    (nc.scalar.copy if cc == 0 else nc.vector.tensor_copy)(f1T[cc][:], pf[:])
                ArawT = spool.tile([BG, TN], bf16, tag="arawT", name="arawT")
                pat = psC()
                for ic in range(4):
                    nc.tensor.transpose(
                        pat[0:BG, ic * 128 : ic * 128 + 128], Araw[ic][:], ident[:])
                nc.vector.tensor_copy(ArawT[:], pat[0:BG, :])

                lf2 = [spool.tile([128, C], bf16, tag=f"lf2_{ic}", name=f"lf2_{ic}")
                       for ic in range(4)]
                lf2pre = [spool.tile([128, C], bf16, tag=f"lf2p{ic}", name=f"lf2p{ic}")
                          for ic in range(4)]
                st2w = spool.tile([128, 24], f32, tag="st2w", name="st2w")
                for ic in range(4):
                    pl = psB()
                    nc.tensor.matmul(
                        pl[:, 0:C], f1T[0][:, ic * 128 : ic * 128 + 128], wafft[0][:],
                        start=True, stop=False)
                    nc.tensor.matmul(
                        pl[:, 0:C], f1T[1][:, ic * 128 : ic * 128 + 128], wafft[1][:],
                        start=False, stop=False)
                    nc.tensor.matmul(
                        pl[:, 0:C], ArawT[0:BG, ic * 128 : ic * 128 + 128],
                        wafft[2][0:BG, :], start=False, stop=False)
                    nc.tensor.matmul(
                        pl[:, 0:C], onesb[0:1, :], baff_b[:],
                        start=False, stop=True)
                    nc.vector.bn_stats(st2w[:, ic * 6 : ic * 6 + 6], pl[:, 0:C])
                    nc.scalar.copy(lf2pre[ic][:], pl[:, 0:C])
                st24 = st2w[:].rearrange("p (i g v) -> p i g v", i=4, g=2)
                mu2 = spool.tile([128, 4], f32, tag="mu2", name="mu2")
                nc.vector.tensor_reduce(
                    mu2[:], st24[:, :, :, 1], axis=AX.X, op=OP.add)
                nc.vector.tensor_scalar_mul(mu2[:], mu2[:], 0.5)
                msq2 = spool.tile([128, 8], f32, tag="msq2", name="msq2")
                msq23 = msq2[:].rearrange("p (i g) -> p i g", i=4)
                nc.vector.tensor_tensor(
                    msq23, st24[:, :, :, 1], st24[:, :, :, 1], op=OP.mult)
                e2t = spool.tile([128, 8], f32, tag="e2t", name="e2t")
                nc.vector.scalar_tensor_tensor(
                    e2t[:].rearrange("p (i g) -> p i g", i=4),
                    st24[:, :, :, 2], 1.0 / 128.0, msq23, op0=OP.mult, op1=OP.add)
                ex2 = spool.tile([128, 4], f32, tag="ex2b", name="ex2b")
                nc.vector.tensor_reduce(
                    ex2[:], e2t[:].rearrange("p (i g) -> p i g", i=4),
                    axis=AX.X, op=OP.add)
                nc.vector.tensor_scalar_mul(ex2[:], ex2[:], 0.5)
                mu2sq = spool.tile([128, 4], f32, tag="mu2sq", name="mu2sq")
                nc.vector.tensor_tensor(mu2sq[:], mu2[:], mu2[:], op=OP.mult)
                nc.vector.tensor_tensor(ex2[:], ex2[:], mu2sq[:], op=OP.subtract)
                nc.vector.tensor_scalar_add(ex2[:], ex2[:], 1e-5)
                sd2 = spool.tile([128, 4], f32, tag="sd2", name="sd2")
                nc.scalar.sqrt(sd2[:], ex2[:])
                al2 = spool.tile([128, 4], f32, tag="al2", name="al2")
                nc.vector.reciprocal(al2[:], sd2[:])
                be2 = spool.tile([128, 4], f32, tag="be2", name="be2")
                nc.vector.scalar_tensor_tensor(
                    be2[:], mu2[:], -1.0, al2[:], op0=OP.mult, op1=OP.mult)
                for ic in range(4):
                    if trivial_gb2 and use_lrelu_act:
                        nc.scalar.activation(
                            lf2[ic][:], lf2pre[ic][:], AF.Lrelu,
                            bias=be2[:, ic : ic + 1], scale=al2[:, ic : ic + 1], alpha=0.01)
                    else:
                        nc.scalar.activation(
                            lf2[ic][:], lf2pre[ic][:], AF.Identity,
                            bias=be2[:, ic : ic + 1], scale=al2[:, ic : ic + 1])
                        if not trivial_gb2:
                            nc.vector.tensor_tensor(lf2[ic][:], lf2[ic][:], g2r[:], op=OP.mult)
                            nc.vector.tensor_tensor(lf2[ic][:], lf2[ic][:], b2r[:], op=OP.add)
                        nc.vector.scalar_tensor_tensor(
                            lf2[ic][:], lf2[ic][:], 0.01, lf2[ic][:],
                            op0=OP.mult, op1=OP.max)

                if stage <= 7:
                    for ic in range(4):
                        scc = spool.tile([128, C], f32, tag="stgc", name="stgc")
                        nc.scalar.copy(scc[:], lf2[ic][:])
                        dma(outb[ic * 128 : ic * 128 + 128, :], scc[:])
                    continue

                # S10: reduce
                catT = [spool.tile([128, TN], bf16, tag=f"catT{j}", name=f"catT{j}")
                        for j in range(4)]
                for cc in range(2):
                    ptx = psC(bf16)
                    for ic in range(4):
                        nc.tensor.transpose(
                            ptx[:, ic * 128 : ic * 128 + 128],
                            lf1b[ic][:, cc * 128 : cc * 128 + 128], identb[:])
                    (nc.scalar.copy if cc == 0 else nc.vector.tensor_copy)(catT[cc][:], ptx[:])
                    ptx2 = psC(bf16)
                    for ic in range(4):
                        nc.tensor.transpose(
                            ptx2[:, ic * 128 : ic * 128 + 128],
                            lf2[ic][:, cc * 128 : cc * 128 + 128], identb[:])
                    (nc.vector.tensor_copy if cc == 0 else nc.scalar.copy)(catT[2 + cc][:], ptx2[:])
                for ic in range(4):
                    pr = psA()
                    for j in range(4):
                        nc.tensor.matmul(
                            pr[:, 0:C], catT[j][:, ic * 128 : ic * 128 + 128], wredt[j][:],
                            start=(j == 0), stop=False)
                    nc.tensor.matmul(
                        pr[:, 0:C], onesb[0:1, :], bred_b[:],
                        start=False, stop=True)
                    if use_lrelu_act:
                        nc.scalar.activation(
                            red_sb[ic][:, b * C : (b + 1) * C], pr[:, 0:C],
                            AF.Lrelu, alpha=0.01)
                    else:
                        nc.scalar.copy(red_sb[ic][:, b * C : (b + 1) * C], pr[:, 0:C])
                        nc.vector.scalar_tensor_tensor(
                            red_sb[ic][:, b * C : (b + 1) * C],
                            red_sb[ic][:, b * C : (b + 1) * C], 0.01,
                            red_sb[ic][:, b * C : (b + 1) * C], op0=OP.mult, op1=OP.max)

                if stage <= 8:
                    for ic in range(4):
                        scc = spool.tile([128, C], f32, tag="stgc", name="stgc")
                        nc.scalar.copy(scc[:], red_sb[ic][:, b * C : (b + 1) * C])
                        dma(outb[ic * 128 : ic * 128 + 128, :], scc[:])
                    continue

                # S11: gate pieces (sigmoid deferred to end)
                str_ = spool.tile([128, 24], f32, tag="strn", name="strn")
                for ic in range(4):
                    nc.vector.bn_stats(
                        str_[:, ic * 6 : ic * 6 + 6], red_sb[ic][:, b * C : (b + 1) * C])
                str4 = str_[:].rearrange("p (i g v) -> p i g v", i=4, g=2)
                msqr = spool.tile([128, 8], f32, tag="msqr", name="msqr")
                msqr3 = msqr[:].rearrange("p (i g) -> p i g", i=4)
                nc.vector.tensor_tensor(
                    msqr3, str4[:, :, :, 1], str4[:, :, :, 1], op=OP.mult)
                m2sr = spool.tile([128, 8], f32, tag="m2sr", name="m2sr")
                nc.vector.scalar_tensor_tensor(
                    m2sr[:].rearrange("p (i g) -> p i g", i=4),
                    msqr3, 128.0, str4[:, :, :, 2], op0=OP.mult, op1=OP.add)
                ssqr = spool.tile([128, 4], f32, tag="ssqr", name="ssqr")
                nc.vector.tensor_reduce(
                    ssqr[:], m2sr[:].rearrange("p (i g) -> p i g", i=4),
                    axis=AX.X, op=OP.add)
                nrmr = spool.tile([128, 4], f32, tag="nrmr", name="nrmr")
                nc.scalar.sqrt(nrmr[:], ssqr[:])
                nc.vector.reciprocal(rinv_w[:, b * 4 : b * 4 + 4], nrmr[:])
                pv = psA()
                for ic in range(4):
                    wa2 = spool.tile([128, 1], bf16, tag="wa2", name="wa2")
                    nc.vector.tensor_tensor(
                        wa2[:], wa_col[:, ic : ic + 1],
                        rinv_w[:, b * 4 + ic : b * 4 + ic + 1], op=OP.mult)
                    nc.tensor.matmul(
                        pv[0:1, 0:C], wa2[:], red_sb[ic][:, b * C : (b + 1) * C],
                        start=(ic == 0), stop=(ic == 3))
                vrow = spool.tile([1, C], bf16, tag="vrow", name="vrow")
                nc.scalar.copy(vrow[:], pv[0:1, 0:C])
                pvr = psA()
                nc.tensor.matmul(pvr[0:128, 0:C], onesb[:], vrow[:], start=True, stop=True)
                vrep = spool.tile([128, C], bf16, tag="vrep", name="vrep")
                nc.vector.tensor_copy(vrep[:], pvr[0:128, 0:C])
                s0w = spool.tile([128, 4], f32, tag="s0w", name="s0w")
                pw0 = spool.tile([128, 4], f32, tag="pw0", name="pw0")
                for ic in range(4):
                    scr = spool.tile([128, C], bf16, tag="gscr", name="gscr")
                    nc.vector.scalar_tensor_tensor(
                        scr[:], red_sb[ic][:, b * C : (b + 1) * C], 1.0, vrep[:],
                        op0=OP.mult, op1=OP.mult,
                        accum_out=s0w[:, ic : ic + 1])
                    scr3 = spool.tile([128, POSD], f32, tag="gscr3", name="gscr3")
                    nc.vector.scalar_tensor_tensor(
                        scr3[:], pos_w[ic][:, b * POSD : (b + 1) * POSD], 1.0, wp_rep[:],
                        op0=OP.mult, op1=OP.mult,
                        accum_out=pw0[:, ic : ic + 1])
                gtmp = spool.tile([128, 4], f32, tag="gtmp", name="gtmp")
                nc.vector.tensor_tensor(
                    gtmp[:], s0w[:], rinv_w[:, b * 4 : b * 4 + 4], op=OP.mult)
                nc.vector.tensor_tensor(gtmp[:], gtmp[:], pw0[:], op=OP.add)
                nc.vector.tensor_tensor(
                    garg_w[:, b * 4 : b * 4 + 4], gtmp[:],
                    batt_rep[:].broadcast_to([128, 4]), op=OP.add)

            # ---------------- end: sigmoid + output ----------------
            if stage > 8:
                att_w16 = wpool.tile([128, 4 * bpc], f32)
                nc.scalar.activation(att_w16[:], garg_w[:], AF.Sigmoid)
                for b in range(bpc):
                    outb = out_d[b].flatten_outer_dims()
                    for ic in range(4):
                        outsb = spool.tile([128, C], f32, tag="outsb", name="outsb")
                        nc.vector.tensor_scalar_mul(
                            outsb[:], red_sb[ic][:, b * C : (b + 1) * C],
                            att_w16[:, b * 4 + ic : b * 4 + ic + 1])
                        dma(outb[ic * 128 : ic * 128 + 128, :], outsb[:])

    nc.finalize()
    return nc


_CACHE = {}


def _get_nc(bpc, trivial_gb1, trivial_gb2, use_lrelu_act=True, stage=9):
    key = (bpc, trivial_gb1, trivial_gb2, use_lrelu_act, stage)
    if key not in _CACHE:
        _CACHE[key] = build_nc(*key)
    return _CACHE[key]


def make_in_maps(inputs, ncores):
    lf = np.asarray(inputs["local_feat"], np.float32)
    gf = np.asarray(inputs["global_feat"], np.float32)
    pos = np.asarray(inputs["pos"], np.float32)
    bpc = lf.shape[0] // ncores

    wcv = np.asarray(inputs["tc_conv_w"], np.float32).reshape(C, KK)
    # Wshift tiles: [idx][row, half*C + c] = wcv[c, 4*jc-2*p-half + row//32] (0 if k out of range)
    wshift = np.zeros((NPAIR, 128, 2 * C), np.float32)
    for (jc, p), i in PAIR_IDX.items():
        for tl in range(4):
            for half in range(2):
                k = 4 * jc - (2 * p + half) + tl
                if 0 <= k < KK:
                    wshift[i, 32 * tl : 32 * tl + 32, half * C : (half + 1) * C] = wcv[:, k][None, :]
    waff = np.asarray(inputs["bi_aff_w"], np.float32)  # (C, C+BG)
    wafft = np.zeros((3, 128, C), np.float32)
    for j in range(3):
        kdim = 128 if j < 2 else BG
        wafft[j, :kdim, :] = waff[:, j * 128 : j * 128 + kdim].T
    redw = np.asarray(inputs["red_w"], np.float32)  # (C, 2C)
    wredt = np.zeros((4, 128, C), np.float32)
    for j in range(4):
        wredt[j] = redw[:, j * 128 : j * 128 + 128].T
    attw = np.asarray(inputs["att_w"], np.float32).reshape(-1)
    wa_col = np.ascontiguousarray(attw[:TN].reshape(4, 128).T)  # (128, 4)
    wp_rep = np.tile(attw[TN : TN + POSD][None, :], (128, 1))
    batt_rep = np.full((128, 1), float(np.asarray(inputs["att_b"]).reshape(-1)[0]), np.float32)
    g1r = np.tile(np.asarray(inputs["tc_ln_g"], np.float32).reshape(1, C), (128, 1))
    b1r = np.tile(np.asarray(inputs["tc_ln_b"], np.float32).reshape(1, C), (128, 1))
    g2r = np.tile(np.asarray(inputs["bi_ln_g"], np.float32).reshape(1, C), (128, 1))
    b2r = np.tile(np.asarray(inputs["bi_ln_b"], np.float32).reshape(1, C), (128, 1))
    ident = np.eye(128, dtype=np.float32)

    params = {
        "tc_adj_w": np.ascontiguousarray(np.asarray(inputs["tc_adj_w"], np.float32)),
        "bi_adj_w": np.ascontiguousarray(np.asarray(inputs["bi_adj_w"], np.float32)),
        "wshift": wshift,
        "wafft": wafft,
        "bi_aff_b": np.asarray(inputs["bi_aff_b"], np.float32).reshape(1, C),
        "wredt": wredt,
        "red_b": np.asarray(inputs["red_b"], np.float32).reshape(1, C),
        "ident": ident,
        "identb": ident,
        "wa_col": wa_col,
        "wp_rep": wp_rep,
        "batt_rep": batt_rep,
        "g1r": g1r, "b1r": b1r, "g2r": g2r, "b2r": b2r,
    }
    in_maps = []
    for core in range(ncores):
        sl = slice(core * bpc, (core + 1) * bpc)
        m = dict(params)
        m["local_feat"] = np.ascontiguousarray(lf[sl])
        m["global_feat"] = np.ascontiguousarray(gf[sl])
        m["pos"] = np.ascontiguousarray(pos[sl])
        in_maps.append(m)
    return in_maps, bpc


def kernel(**inputs):
    from concourse.bass_utils import run_bass_kernel_spmd

    trivial_gb1 = bool(
        np.allclose(inputs["tc_ln_g"], 1.0) and np.allclose(inputs["tc_ln_b"], 0.0)
    )
    trivial_gb2 = bool(
        np.allclose(inputs["bi_ln_g"], 1.0) and np.allclose(inputs["bi_ln_b"], 0.0)
    )
    in_maps, bpc = make_in_maps(inputs, NCORES)
    nc = _get_nc(bpc, trivial_gb1, trivial_gb2)
    res = run_bass_kernel_spmd(nc, in_maps, core_ids=list(range(NCORES)))
    outs = [res.results[c]["out"] for c in range(NCORES)]
    return np.concatenate(outs, axis=0).reshape(B, T, N, C)


if __name__ == "__main__":
    nc = build_nc(1, True, True)
    print("build ok")
